# revision 4
# baseline (speedup 1.0000x reference)
"""Causal attention (softmax over query axis) on 8 trn2 NeuronCores.

Problem: x[4,2048,1024], W_q/W_k/W_v[1024,1024] (fp32)
  q,k,v = x@W_q, x@W_k, x@W_v
  scores[b,i,j] = q_i.k_j ; masked -inf where j>i ; scaled by 1/32
  weights = softmax(scores, axis=1)   # over the QUERY axis i (faithful quirk)
  out = weights @ v

Sharding: 8 cores = 4 batches x 2 halves of the output-feature dim.
Core c: batch b=c//2, e-half h=c%2. Every core runs an IDENTICAL program:
  - full Q^T, K^T (needed for full-column softmax stats), V for its 512-col
    half (W_v pre-sliced host-side - pure data parallelism),
  - scores computed transposed ST[j,i]=k_j.q_i so the softmax (over i) is a
    free-dim row softmax; causal-valid region is i>=j (row-aligned triangle),
  - A = exp((ST-max)/32) (triangle, fp32r), rowsum -> rinv folded into V,
  - out half = A^T @ (V*rinv) accumulated over j-tiles in PSUM.
Host concatenates the two e-halves per batch.

All matmuls run in fp32r (1 cycle/row at N>=512, ~1.7e-4 rel err).
"""

import math
from contextlib import ExitStack

import numpy as np

B, S, D = 4, 2048, 1024
P = 128
NT = S // P        # 16 j/i tiles
ET = D // P        # 8 e/d tiles
EH = 512           # e-half width
CH = 512           # score chunk width
INV_SQRT_D = 1.0 / math.sqrt(D)
NEG = -1e30

# row jt of the exp-triangle starts at free-offset OFFS[jt], length 2048-128*jt
ROWLEN = [S - P * jt for jt in range(NT)]
OFFS = np.concatenate([[0], np.cumsum(ROWLEN)]).tolist()
A_TOTAL = OFFS[NT]  # 17408


def build_program():
    import concourse.tile as tile
    from concourse import bacc, mybir
    from concourse.masks import make_identity

    f32 = mybir.dt.float32
    f32r = mybir.dt.float32r
    AX = mybir.AxisListType
    OP = mybir.AluOpType
    AF = mybir.ActivationFunctionType

    nc = bacc.Bacc("TRN2", target_bir_lowering=False, debug=False,
                   enable_asserts=False, num_devices=8)

    x_ap = nc.dram_tensor("x", [S, D], f32, kind="ExternalInput").ap()
    wq_ap = nc.dram_tensor("wq", [D, D], f32, kind="ExternalInput").ap()
    wk_ap = nc.dram_tensor("wk", [D, D], f32, kind="ExternalInput").ap()
    wvh_ap = nc.dram_tensor("wvh", [D, EH], f32, kind="ExternalInput").ap()
    out_ap = nc.dram_tensor("out", [S, EH], f32, kind="ExternalOutput").ap()

    with tile.TileContext(nc) as tc, ExitStack() as ctx:
        pool = lambda name, bufs, **kw: ctx.enter_context(
            tc.tile_pool(name=name, bufs=bufs, **kw))

        const = pool("const", 1)
        stats = pool("stats", 4)
        dram = pool("dram", 1, space="DRAM")
        qt_pool = pool("qt", 1)
        vh_pool = pool("vh", 1)

        ident = const.tile([P, P], f32)
        make_identity(nc, ident[:])
        # additive causal mask for the diagonal block: 0 where x>=p else NEG
        mask = const.tile([P, P], f32)
        nc.gpsimd.memset(mask[:], 0.0)
        nc.gpsimd.affine_select(
            out=mask[:], in_=mask[:], compare_op=OP.is_ge, fill=NEG,
            base=0, pattern=[[1, P]], channel_multiplier=-1)

        # persistent big tensors
        QT = [qt_pool.tile([P, S], f32r, tag=f"qt{eb}", name=f"qt{eb}") for eb in range(ET)]
        Vh = [vh_pool.tile([P, EH], f32r, tag=f"vh{jb}", name=f"vh{jb}") for jb in range(NT)]
        # K^T spill: [jt, eb, p(e within block), j] in scratch DRAM
        KT_dram = dram.tile([NT, ET, P, P], f32r)

        with tc.tile_pool(name="xt", bufs=1) as xt_pool:
            XT = [xt_pool.tile([P, S], f32r, tag=f"xt{dt}", name=f"xt{dt}") for dt in range(ET)]

            with (tc.tile_pool(name="tps", bufs=4, space="PSUM") as tps,
                  tc.tile_pool(name="pps", bufs=4, space="PSUM") as pps):

                # ---- Phase 1: xT (PE transpose of x) ----
                with tc.tile_pool(name="xrows", bufs=3) as xrows:
                  for st in range(NT):
                    xrow = xrows.tile([P, D], f32, tag="xrow")
                    nc.sync.dma_start(xrow[:], x_ap[P * st:P * (st + 1), :])
                    for dt in range(ET):
                        ps = tps.tile([P, P], f32, tag="tr")
                        nc.tensor.transpose(ps[:], xrow[:, P * dt:P * (dt + 1)], ident[:])
                        nc.scalar.copy(XT[dt][:, P * st:P * (st + 1)], ps[:])

                # ---- Phase 2: K^T -> DRAM spill ----
                with tc.tile_pool(name="wk", bufs=1) as wkp, \
                     tc.tile_pool(name="p2s", bufs=2) as p2s:
                    WK = [wkp.tile([P, D], f32r, tag=f"wk{dt}", name=f"wk{dt}") for dt in range(ET)]
                    for dt in range(ET):
                        wrow = p2s.tile([P, D], f32, tag="wrow")
                        nc.sync.dma_start(wrow[:], wk_ap[P * dt:P * (dt + 1), :])
                        nc.vector.tensor_copy(WK[dt][:], wrow[:])
                    for eb in range(ET):
                        for jc in range(S // CH):
                            ps = pps.tile([P, CH], f32, tag="proj")
                            for dt in range(ET):
                                nc.tensor.matmul(
                                    ps[:], WK[dt][:, P * eb:P * (eb + 1)],
                                    XT[dt][:, CH * jc:CH * (jc + 1)],
                                    start=(dt == 0), stop=(dt == ET - 1))
                            kt_sb = p2s.tile([P, CH], f32r, tag="ktev")
                            nc.scalar.copy(kt_sb[:], ps[:])
                            # dst iteration (jt', p, j): KT_dram[4jc+jt', eb, p, j]
                            dst = KT_dram[4 * jc:4 * (jc + 1), eb, :, :].rearrange(
                                "t p j -> p t j")
                            nc.sync.dma_start(dst, kt_sb[:].rearrange(
                                "p (t j) -> p t j", j=P))

                # ---- Phase 3: Q^T (resident) ----
                with tc.tile_pool(name="wq", bufs=1) as wqp, \
                     tc.tile_pool(name="p3s", bufs=2) as p3s:
                    WQ = [wqp.tile([P, D], f32r, tag=f"wq{dt}", name=f"wq{dt}") for dt in range(ET)]
                    for dt in range(ET):
                        wrow = p3s.tile([P, D], f32, tag="wrow")
                        nc.sync.dma_start(wrow[:], wq_ap[P * dt:P * (dt + 1), :])
                        nc.vector.tensor_copy(WQ[dt][:], wrow[:])
                    for eb in range(ET):
                        for ic in range(S // CH):
                            ps = pps.tile([P, CH], f32, tag="proj")
                            for dt in range(ET):
                                nc.tensor.matmul(
                                    ps[:], WQ[dt][:, P * eb:P * (eb + 1)],
                                    XT[dt][:, CH * ic:CH * (ic + 1)],
                                    start=(dt == 0), stop=(dt == ET - 1))
                            nc.scalar.copy(QT[eb][:, CH * ic:CH * (ic + 1)], ps[:])

                # ---- Phase 4: V half (resident) ----
                with tc.tile_pool(name="wv", bufs=1) as wvp, \
                     tc.tile_pool(name="p4s", bufs=2) as p4s:
                    WV = [wvp.tile([P, EH], f32r, tag=f"wv{dt}", name=f"wv{dt}") for dt in range(ET)]
                    for dt in range(ET):
                        wrow = p4s.tile([P, EH], f32, tag="wvrow")
                        nc.sync.dma_start(wrow[:], wvh_ap[P * dt:P * (dt + 1), :])
                        nc.vector.tensor_copy(WV[dt][:], wrow[:])
                    for jb in range(NT):
                        ps = pps.tile([P, EH], f32, tag="proj")
                        for dt in range(ET):
                            nc.tensor.matmul(
                                ps[:], XT[dt][:, P * jb:P * (jb + 1)], WV[dt][:],
                                start=(dt == 0), stop=(dt == ET - 1))
                        nc.scalar.copy(Vh[jb][:], ps[:])

        # ---- Phase 5: scores triangle + softmax stats ----
        with tc.tile_pool(name="apool", bufs=1) as apool:
            A = apool.tile([P, A_TOTAL], f32r)

            with tc.tile_pool(name="sps", bufs=8, space="PSUM") as sps, \
                 tc.tile_pool(name="p5s", bufs=2) as p5s:
                for jt in range(NT):
                    kt = p5s.tile([P, ET, P], f32r, tag="ktld")
                    nc.sync.dma_start(
                        kt[:], KT_dram[jt, :, :, :].rearrange("e p j -> p e j"))
                    istart = P * jt
                    rl = ROWLEN[jt]
                    nch = (rl + CH - 1) // CH
                    chunks = []
                    for k in range(nch):
                        w = min(CH, rl - CH * k)
                        ps = sps.tile([P, CH], f32, tag="sc")
                        for eb in range(ET):
                            nc.tensor.matmul(
                                ps[:, :w], kt[:, eb, :],
                                QT[eb][:, istart + CH * k: istart + CH * k + w],
                                start=(eb == 0), stop=(eb == ET - 1))
                        chunks.append((ps, w))
                    # causal mask on the diagonal 128 block (in place in PSUM)
                    ps0 = chunks[0][0]
                    nc.vector.tensor_add(ps0[:, 0:P], ps0[:, 0:P], mask[:])
                    # row max over the full valid row
                    m = stats.tile([P, 1], f32, tag="m")
                    for k, (ps, w) in enumerate(chunks):
                        cm = stats.tile([P, 1], f32, tag="cm")
                        nc.vector.tensor_reduce(cm[:], ps[:, :w], axis=AX.X, op=OP.max)
                        if k == 0:
                            nc.vector.tensor_copy(m[:], cm[:])
                        else:
                            nc.vector.tensor_max(m[:], m[:], cm[:])
                    nbias = stats.tile([P, 1], f32, tag="nb")
                    nc.scalar.mul(nbias[:], m[:], -INV_SQRT_D)
                    # exp((s - m)/32) -> A row (f32r) + fused row-sum
                    rsum = stats.tile([P, 1], f32, tag="rs")
                    for k, (ps, w) in enumerate(chunks):
                        cs = stats.tile([P, 1], f32, tag="cs")
                        nc.scalar.activation(
                            A[:, OFFS[jt] + CH * k: OFFS[jt] + CH * k + w],
                            ps[:, :w], AF.Exp,
                            bias=nbias[:], scale=INV_SQRT_D, accum_out=cs[:])
                        if k == 0:
                            nc.vector.tensor_copy(rsum[:], cs[:])
                        else:
                            nc.vector.tensor_add(rsum[:], rsum[:], cs[:])
                    rinv = stats.tile([P, 1], f32, tag="ri")
                    nc.vector.reciprocal(rinv[:], rsum[:])
                    # fold 1/rowsum into V: V'[jt] = V[jt] * rinv_j
                    nc.vector.tensor_scalar_mul(Vh[jt][:], Vh[jt][:], rinv[:])

            # ---- Phase 6: out half = A^T @ V' ----
            with tc.tile_pool(name="avps", bufs=3, space="PSUM") as avps, \
                 tc.tile_pool(name="p6s", bufs=3) as p6s:
                for it in range(NT):
                    ps = avps.tile([P, EH], f32, tag="av")
                    for jt in range(it + 1):
                        nc.tensor.matmul(
                            ps[:], A[:, OFFS[jt] + P * (it - jt): OFFS[jt] + P * (it - jt + 1)],
                            Vh[jt][:], start=(jt == 0), stop=(jt == it))
                    o_sb = p6s.tile([P, EH], f32, tag="oev")
                    nc.scalar.copy(o_sb[:], ps[:])
                    nc.sync.dma_start(out_ap[P * it:P * (it + 1), :], o_sb[:])

    nc.compile()
    return nc


_PROGRAM_CACHE = {}


def kernel(x, W_q, W_k, W_v):
    from concourse.bass_utils import run_bass_kernel_spmd

    x = np.asarray(x, dtype=np.float32)
    W_q = np.asarray(W_q, dtype=np.float32)
    W_k = np.asarray(W_k, dtype=np.float32)
    W_v = np.asarray(W_v, dtype=np.float32)

    if "nc" not in _PROGRAM_CACHE:
        _PROGRAM_CACHE["nc"] = build_program()
    nc = _PROGRAM_CACHE["nc"]

    in_maps = []
    for c in range(8):
        b, h = c // 2, c % 2
        in_maps.append({
            "x": np.ascontiguousarray(x[b]),
            "wq": W_q,
            "wk": W_k,
            "wvh": np.ascontiguousarray(W_v[:, h * EH:(h + 1) * EH]),
        })

    res = run_bass_kernel_spmd(nc, in_maps, core_ids=list(range(8)))
    out = np.empty((B, S, D), dtype=np.float32)
    for c in range(8):
        b, h = c // 2, c % 2
        out[b, :, h * EH:(h + 1) * EH] = res.results[c]["out"]
    return out


# revision 6
# speedup vs baseline: 105.6016x; 105.6016x over previous
"""Causal attention (softmax over query axis) on 8 trn2 NeuronCores.

Problem: x[4,2048,1024], W_q/W_k/W_v[1024,1024] (fp32)
  q,k,v = x@W_q, x@W_k, x@W_v
  scores[b,i,j] = q_i.k_j ; masked -inf where j>i ; scaled by 1/32
  weights = softmax(scores, axis=1)   # over the QUERY axis i (faithful quirk)
  out = weights @ v

Sharding: 8 cores = 4 batches x 2 halves of the output-feature dim.
Core c: batch b=c//2, e-half h=c%2. Every core runs an IDENTICAL program:
  - full Q^T, K^T (needed for full-column softmax stats), V for its 512-col
    half (W_v pre-sliced host-side - pure data parallelism),
  - scores computed transposed ST[j,i]=k_j.q_i so the softmax (over i) is a
    free-dim row softmax; causal-valid region is i>=j (row-aligned triangle),
  - A = exp((ST-max)/32) (triangle, fp32r), rowsum -> rinv folded into V,
  - out half = A^T @ (V*rinv) accumulated over j-tiles in PSUM.
Host concatenates the two e-halves per batch.

All matmuls run in fp32r (1 cycle/row at N>=256, ~1.5e-4 rel err); raw fp32
bits are declared f32r at the DRAM boundary (validated on HW). x^T is
supplied pre-transposed by the host (free numpy prep in kernel()).
"""

import math
from contextlib import ExitStack

import numpy as np

B, S, D = 4, 2048, 1024
P = 128
NT = S // P        # 16 j/i tiles
ET = D // P        # 8 e/d tiles
EH = 512           # e-half width
CH = 512           # score chunk width
INV_SQRT_D = 1.0 / math.sqrt(D)
NEG = -1e30

# row jt of the exp-triangle starts at free-offset OFFS[jt], length 2048-128*jt
ROWLEN = [S - P * jt for jt in range(NT)]
OFFS = np.concatenate([[0], np.cumsum(ROWLEN)]).tolist()
A_TOTAL = OFFS[NT]  # 17408


def chunk_widths(rl):
    """Split a score row of length rl into matmul chunks <=512, avoiding
    width-128 chunks (fp32r runs 4x slower below N=256)."""
    nch = (rl + CH - 1) // CH
    ws = [CH] * (rl // CH)
    rem = rl - CH * (rl // CH)
    if rem == P and ws:
        ws[-1] = 384
        ws.append(256)
    elif rem:
        ws.append(rem)
    assert sum(ws) == rl
    return ws


def build_program():
    import concourse.tile as tile
    from concourse import bacc, mybir

    f32 = mybir.dt.float32
    f32r = mybir.dt.float32r
    AX = mybir.AxisListType
    OP = mybir.AluOpType
    AF = mybir.ActivationFunctionType

    nc = bacc.Bacc("TRN2", target_bir_lowering=False, debug=False,
                   enable_asserts=False, num_devices=8)

    xt_ap = nc.dram_tensor("xt", [D, S], f32r, kind="ExternalInput").ap()
    wq_ap = nc.dram_tensor("wq", [D, D], f32r, kind="ExternalInput").ap()
    wk_ap = nc.dram_tensor("wk", [D, D], f32r, kind="ExternalInput").ap()
    wvh_ap = nc.dram_tensor("wvh", [D, EH], f32r, kind="ExternalInput").ap()
    out_ap = nc.dram_tensor("out", [S, EH], f32, kind="ExternalOutput").ap()

    with tile.TileContext(nc) as tc, ExitStack() as ctx:
        pool = lambda name, bufs, **kw: ctx.enter_context(
            tc.tile_pool(name=name, bufs=bufs, **kw))

        const = pool("const", 1)
        stats = pool("stats", 4)
        dram = pool("dram", 1, space="DRAM")
        qt_pool = pool("qt", 1)
        vh_pool = pool("vh", 1)

        # additive causal mask for the diagonal block: 0 where x>=p else NEG
        mask = const.tile([P, P], f32)
        nc.gpsimd.memset(mask[:], 0.0)
        nc.gpsimd.affine_select(
            out=mask[:], in_=mask[:], compare_op=OP.is_ge, fill=NEG,
            base=0, pattern=[[1, P]], channel_multiplier=-1)

        # persistent big tensors
        QT = [qt_pool.tile([P, S], f32r, tag=f"qt{eb}", name=f"qt{eb}") for eb in range(ET)]
        Vh = [vh_pool.tile([P, EH], f32r, tag=f"vh{jb}", name=f"vh{jb}") for jb in range(NT)]
        # K^T spill: [jt, eb, p(e within block), j] in scratch DRAM
        KT_dram = dram.tile([NT, ET, P, P], f32r)

        with tc.tile_pool(name="xt", bufs=1) as xt_pool:
            XT = [xt_pool.tile([P, S], f32r, tag=f"xt{dt}", name=f"xt{dt}") for dt in range(ET)]

            # ---- Phase 1: xT loaded directly (host supplies x transposed) ----
            for dt in range(ET):
                nc.sync.dma_start(XT[dt][:], xt_ap[P * dt:P * (dt + 1), :])

            with tc.tile_pool(name="pps", bufs=6, space="PSUM") as pps:

                # ---- Phase 2: K^T -> DRAM spill ----
                with tc.tile_pool(name="wk", bufs=1) as wkp, \
                     tc.tile_pool(name="p2s", bufs=3) as p2s:
                    WK = [wkp.tile([P, D], f32r, tag=f"wk{dt}", name=f"wk{dt}") for dt in range(ET)]
                    for dt in range(ET):
                        nc.sync.dma_start(WK[dt][:], wk_ap[P * dt:P * (dt + 1), :])
                    for eb in range(ET):
                        for jc in range(S // CH):
                            ps = pps.tile([P, CH], f32, tag="proj")
                            for dt in range(ET):
                                nc.tensor.matmul(
                                    ps[:], WK[dt][:, P * eb:P * (eb + 1)],
                                    XT[dt][:, CH * jc:CH * (jc + 1)],
                                    start=(dt == 0), stop=(dt == ET - 1))
                            kt_sb = p2s.tile([P, CH], f32r, tag="ktev")
                            nc.vector.tensor_copy(kt_sb[:], ps[:])
                            # dst iteration (jt', p, j): KT_dram[4jc+jt', eb, p, j]
                            dst = KT_dram[4 * jc:4 * (jc + 1), eb, :, :].rearrange(
                                "t p j -> p t j")
                            nc.sync.dma_start(dst, kt_sb[:].rearrange(
                                "p (t j) -> p t j", j=P))

                # ---- Phase 3: Q^T (resident) ----
                with tc.tile_pool(name="wq", bufs=1) as wqp:
                    WQ = [wqp.tile([P, D], f32r, tag=f"wq{dt}", name=f"wq{dt}") for dt in range(ET)]
                    for dt in range(ET):
                        nc.sync.dma_start(WQ[dt][:], wq_ap[P * dt:P * (dt + 1), :])
                    for eb in range(ET):
                        for ic in range(S // CH):
                            ps = pps.tile([P, CH], f32, tag="proj")
                            for dt in range(ET):
                                nc.tensor.matmul(
                                    ps[:], WQ[dt][:, P * eb:P * (eb + 1)],
                                    XT[dt][:, CH * ic:CH * (ic + 1)],
                                    start=(dt == 0), stop=(dt == ET - 1))
                            nc.scalar.copy(QT[eb][:, CH * ic:CH * (ic + 1)], ps[:])

                # ---- Phase 4: V half (resident) ----
                with tc.tile_pool(name="wv", bufs=1) as wvp:
                    WV = [wvp.tile([P, EH], f32r, tag=f"wv{dt}", name=f"wv{dt}") for dt in range(ET)]
                    for dt in range(ET):
                        nc.sync.dma_start(WV[dt][:], wvh_ap[P * dt:P * (dt + 1), :])
                    for jb in range(NT):
                        ps = pps.tile([P, EH], f32, tag="proj")
                        for dt in range(ET):
                            nc.tensor.matmul(
                                ps[:], XT[dt][:, P * jb:P * (jb + 1)], WV[dt][:],
                                start=(dt == 0), stop=(dt == ET - 1))
                        nc.vector.tensor_copy(Vh[jb][:], ps[:])

        # ---- Phase 5: scores triangle + softmax stats ----
        with tc.tile_pool(name="apool", bufs=1) as apool:
            A = apool.tile([P, A_TOTAL], f32r)

            with tc.tile_pool(name="sps", bufs=8, space="PSUM") as sps, \
                 tc.tile_pool(name="p5s", bufs=2) as p5s:
                for jt in range(NT):
                    kt = p5s.tile([P, ET, P], f32r, tag="ktld")
                    nc.sync.dma_start(
                        kt[:], KT_dram[jt, :, :, :].rearrange("e p j -> p e j"))
                    istart = P * jt
                    chunks = []
                    off = 0
                    for w in chunk_widths(ROWLEN[jt]):
                        ps = sps.tile([P, CH], f32, tag="sc")
                        for eb in range(ET):
                            nc.tensor.matmul(
                                ps[:, :w], kt[:, eb, :],
                                QT[eb][:, istart + off: istart + off + w],
                                start=(eb == 0), stop=(eb == ET - 1))
                        chunks.append((ps, w, off))
                        off += w
                    # causal mask on the diagonal 128 block (in place in PSUM)
                    ps0 = chunks[0][0]
                    nc.vector.tensor_add(ps0[:, 0:P], ps0[:, 0:P], mask[:])
                    # row max over the full valid row
                    m = stats.tile([P, 1], f32, tag="m")
                    for k, (ps, w, off) in enumerate(chunks):
                        cm = stats.tile([P, 1], f32, tag="cm")
                        nc.vector.tensor_reduce(cm[:], ps[:, :w], axis=AX.X, op=OP.max)
                        if k == 0:
                            nc.vector.tensor_copy(m[:], cm[:])
                        else:
                            nc.vector.tensor_max(m[:], m[:], cm[:])
                    nbias = stats.tile([P, 1], f32, tag="nb")
                    nc.scalar.mul(nbias[:], m[:], -INV_SQRT_D)
                    # exp((s - m)/32) -> A row (f32r) + fused row-sum
                    rsum = stats.tile([P, 1], f32, tag="rs")
                    for k, (ps, w, off) in enumerate(chunks):
                        cs = stats.tile([P, 1], f32, tag="cs")
                        nc.scalar.activation(
                            A[:, OFFS[jt] + off: OFFS[jt] + off + w],
                            ps[:, :w], AF.Exp,
                            bias=nbias[:], scale=INV_SQRT_D, accum_out=cs[:])
                        if k == 0:
                            nc.vector.tensor_copy(rsum[:], cs[:])
                        else:
                            nc.vector.tensor_add(rsum[:], rsum[:], cs[:])
                    rinv = stats.tile([P, 1], f32, tag="ri")
                    nc.vector.reciprocal(rinv[:], rsum[:])
                    # fold 1/rowsum into V: V'[jt] = V[jt] * rinv_j
                    nc.vector.tensor_scalar_mul(Vh[jt][:], Vh[jt][:], rinv[:])

            # ---- Phase 6: out half = A^T @ V' ----
            with tc.tile_pool(name="avps", bufs=3, space="PSUM") as avps, \
                 tc.tile_pool(name="p6s", bufs=3) as p6s:
                for it in range(NT):
                    ps = avps.tile([P, EH], f32, tag="av")
                    for jt in range(it + 1):
                        nc.tensor.matmul(
                            ps[:], A[:, OFFS[jt] + P * (it - jt): OFFS[jt] + P * (it - jt + 1)],
                            Vh[jt][:], start=(jt == 0), stop=(jt == it))
                    o_sb = p6s.tile([P, EH], f32, tag="oev")
                    nc.scalar.copy(o_sb[:], ps[:])
                    nc.sync.dma_start(out_ap[P * it:P * (it + 1), :], o_sb[:])

    nc.compile()
    return nc


_PROGRAM_CACHE = {}


def kernel(x, W_q, W_k, W_v):
    from concourse.bass_utils import run_bass_kernel_spmd

    x = np.asarray(x, dtype=np.float32)
    W_q = np.asarray(W_q, dtype=np.float32)
    W_k = np.asarray(W_k, dtype=np.float32)
    W_v = np.asarray(W_v, dtype=np.float32)

    if "nc" not in _PROGRAM_CACHE:
        _PROGRAM_CACHE["nc"] = build_program()
    nc = _PROGRAM_CACHE["nc"]

    in_maps = []
    for c in range(8):
        b, h = c // 2, c % 2
        in_maps.append({
            "xt": np.ascontiguousarray(x[b].T),
            "wq": W_q,
            "wk": W_k,
            "wvh": np.ascontiguousarray(W_v[:, h * EH:(h + 1) * EH]),
        })

    res = run_bass_kernel_spmd(nc, in_maps, core_ids=list(range(8)))
    out = np.empty((B, S, D), dtype=np.float32)
    for c in range(8):
        b, h = c // 2, c % 2
        out[b, :, h * EH:(h + 1) * EH] = res.results[c]["out"]
    return out


# revision 7
# speedup vs baseline: 124.9410x; 1.1831x over previous
"""Causal attention (softmax over query axis) on 8 trn2 NeuronCores.

Problem: x[4,2048,1024], W_q/W_k/W_v[1024,1024] (fp32)
  q,k,v = x@W_q, x@W_k, x@W_v
  scores[b,i,j] = q_i.k_j ; masked -inf where j>i ; scaled by 1/32
  weights = softmax(scores, axis=1)   # over the QUERY axis i (faithful quirk)
  out = weights @ v

Sharding: 8 cores = 4 batches x 2 halves of the output-feature dim.
Core c: batch b=c//2, e-half h=c%2. Every core runs an IDENTICAL program:
  - full Q^T, K^T (needed for full-column softmax stats), V for its 512-col
    half (W_v pre-sliced host-side - pure data parallelism),
  - scores computed transposed ST[j,i]=k_j.q_i so the softmax (over i) is a
    free-dim row softmax; causal-valid region is i>=j (row-aligned triangle),
  - A = exp((ST-max)/32) (triangle, fp32r), rowsum -> rinv folded into V,
  - out half = A^T @ (V*rinv) accumulated over j-tiles in PSUM.
Host concatenates the two e-halves per batch.

All matmuls run in fp32r (1 cycle/row at N>=256, ~1.5e-4 rel err); raw fp32
bits are declared f32r at the DRAM boundary (validated on HW). x^T is
supplied pre-transposed by the host (free numpy prep in kernel()).
"""

import math
from contextlib import ExitStack

import numpy as np

B, S, D = 4, 2048, 1024
P = 128
NT = S // P        # 16 j/i tiles
ET = D // P        # 8 e/d tiles
EH = 512           # e-half width
CH = 512           # score chunk width
INV_SQRT_D = 1.0 / math.sqrt(D)
NEG = -1e30

# row jt of the exp-triangle starts at free-offset OFFS[jt], length 2048-128*jt
ROWLEN = [S - P * jt for jt in range(NT)]
OFFS = np.concatenate([[0], np.cumsum(ROWLEN)]).tolist()
A_TOTAL = OFFS[NT]  # 17408


def chunk_widths(rl):
    """Split a score row of length rl into matmul chunks <=512, avoiding
    width-128 chunks (fp32r runs 4x slower below N=256)."""
    nch = (rl + CH - 1) // CH
    ws = [CH] * (rl // CH)
    rem = rl - CH * (rl // CH)
    if rem == P and ws:
        ws[-1] = 384
        ws.append(256)
    elif rem:
        ws.append(rem)
    assert sum(ws) == rl
    return ws


def build_program():
    import concourse.tile as tile
    from concourse import bacc, mybir

    f32 = mybir.dt.float32
    f32r = mybir.dt.float32r
    AX = mybir.AxisListType
    OP = mybir.AluOpType
    AF = mybir.ActivationFunctionType

    nc = bacc.Bacc("TRN2", target_bir_lowering=False, debug=False,
                   enable_asserts=False, num_devices=8)

    xt_ap = nc.dram_tensor("xt", [D, S], f32r, kind="ExternalInput").ap()
    wq_ap = nc.dram_tensor("wq", [D, D], f32r, kind="ExternalInput").ap()
    wk_ap = nc.dram_tensor("wk", [D, D], f32r, kind="ExternalInput").ap()
    wvh_ap = nc.dram_tensor("wvh", [D, EH], f32r, kind="ExternalInput").ap()
    out_ap = nc.dram_tensor("out", [S, EH], f32, kind="ExternalOutput").ap()

    with tile.TileContext(nc) as tc, ExitStack() as ctx:
        pool = lambda name, bufs, **kw: ctx.enter_context(
            tc.tile_pool(name=name, bufs=bufs, **kw))

        const = pool("const", 1)
        stats = pool("stats", 4)
        dram = pool("dram", 1, space="DRAM")
        qt_pool = pool("qt", 1)
        vh_pool = pool("vh", 1)

        # additive causal mask for the diagonal block: 0 where x>=p else NEG
        mask = const.tile([P, P], f32)
        nc.gpsimd.memset(mask[:], 0.0)
        nc.gpsimd.affine_select(
            out=mask[:], in_=mask[:], compare_op=OP.is_ge, fill=NEG,
            base=0, pattern=[[1, P]], channel_multiplier=-1)

        # persistent big tensors
        QT = [qt_pool.tile([P, S], f32r, tag=f"qt{eb}", name=f"qt{eb}") for eb in range(ET)]
        Vh = [vh_pool.tile([P, EH], f32r, tag=f"vh{jb}", name=f"vh{jb}") for jb in range(NT)]
        # K^T spill: [jt, eb, p(e within block), j] in scratch DRAM
        KT_dram = dram.tile([NT, ET, P, P], f32r)

        with tc.tile_pool(name="xt", bufs=1) as xt_pool:
            XT = [xt_pool.tile([P, S], f32r, tag=f"xt{dt}", name=f"xt{dt}") for dt in range(ET)]

            # ---- Phase 1: xT loaded directly (host supplies x transposed) ----
            for dt in range(ET):
                nc.sync.dma_start(XT[dt][:], xt_ap[P * dt:P * (dt + 1), :])

            with tc.tile_pool(name="pps", bufs=8, space="PSUM") as pps:

                # ---- Phase 2: K^T -> DRAM spill ----
                with tc.tile_pool(name="wk", bufs=1) as wkp, \
                     tc.tile_pool(name="p2s", bufs=3) as p2s:
                    WK = [wkp.tile([P, D], f32r, tag=f"wk{dt}", name=f"wk{dt}") for dt in range(ET)]
                    for dt in range(ET):
                        nc.sync.dma_start(WK[dt][:], wk_ap[P * dt:P * (dt + 1), :])
                    for eb in range(ET):
                        for jc in range(S // CH):
                            ps = pps.tile([P, CH], f32, tag="proj")
                            for dt in range(ET):
                                nc.tensor.matmul(
                                    ps[:], WK[dt][:, P * eb:P * (eb + 1)],
                                    XT[dt][:, CH * jc:CH * (jc + 1)],
                                    start=(dt == 0), stop=(dt == ET - 1))
                            kt_sb = p2s.tile([P, CH], f32r, tag="ktev")
                            nc.vector.tensor_copy(kt_sb[:], ps[:])
                            # dst iteration (jt', p, j): KT_dram[4jc+jt', eb, p, j]
                            dst = KT_dram[4 * jc:4 * (jc + 1), eb, :, :].rearrange(
                                "t p j -> p t j")
                            nc.sync.dma_start(dst, kt_sb[:].rearrange(
                                "p (t j) -> p t j", j=P))

                # ---- Phase 3: Q^T (resident) ----
                with tc.tile_pool(name="wq", bufs=1) as wqp:
                    WQ = [wqp.tile([P, D], f32r, tag=f"wq{dt}", name=f"wq{dt}") for dt in range(ET)]
                    for dt in range(ET):
                        nc.sync.dma_start(WQ[dt][:], wq_ap[P * dt:P * (dt + 1), :])
                    for eb in range(ET):
                        for ic in range(S // CH):
                            ps = pps.tile([P, CH], f32, tag="proj")
                            for dt in range(ET):
                                nc.tensor.matmul(
                                    ps[:], WQ[dt][:, P * eb:P * (eb + 1)],
                                    XT[dt][:, CH * ic:CH * (ic + 1)],
                                    start=(dt == 0), stop=(dt == ET - 1))
                            nc.scalar.copy(QT[eb][:, CH * ic:CH * (ic + 1)], ps[:])

                # ---- Phase 4: V half (resident) ----
                with tc.tile_pool(name="wv", bufs=1) as wvp:
                    WV = [wvp.tile([P, EH], f32r, tag=f"wv{dt}", name=f"wv{dt}") for dt in range(ET)]
                    for dt in range(ET):
                        nc.sync.dma_start(WV[dt][:], wvh_ap[P * dt:P * (dt + 1), :])
                    for jb in range(NT):
                        ps = pps.tile([P, EH], f32, tag="proj")
                        for dt in range(ET):
                            nc.tensor.matmul(
                                ps[:], XT[dt][:, P * jb:P * (jb + 1)], WV[dt][:],
                                start=(dt == 0), stop=(dt == ET - 1))
                        nc.vector.tensor_copy(Vh[jb][:], ps[:])

        # ---- Phase 5: scores triangle + softmax stats ----
        with tc.tile_pool(name="apool", bufs=1) as apool:
            A = apool.tile([P, A_TOTAL], f32r)

            with tc.tile_pool(name="sps", bufs=8, space="PSUM") as sps, \
                 tc.tile_pool(name="p5s", bufs=3) as p5s:
                for jt in range(NT):
                    kt = p5s.tile([P, ET, P], f32r, tag="ktld")
                    nc.sync.dma_start(
                        kt[:], KT_dram[jt, :, :, :].rearrange("e p j -> p e j"))
                    istart = P * jt
                    chunks = []
                    off = 0
                    for w in chunk_widths(ROWLEN[jt]):
                        ps = sps.tile([P, CH], f32, tag="sc")
                        for eb in range(ET):
                            nc.tensor.matmul(
                                ps[:, :w], kt[:, eb, :],
                                QT[eb][:, istart + off: istart + off + w],
                                start=(eb == 0), stop=(eb == ET - 1))
                        chunks.append((ps, w, off))
                        off += w
                    # causal mask on the diagonal 128 block (in place in PSUM)
                    ps0 = chunks[0][0]
                    nc.vector.tensor_add(ps0[:, 0:P], ps0[:, 0:P], mask[:])
                    # row max over the full valid row
                    m = stats.tile([P, 1], f32, tag="m")
                    for k, (ps, w, off) in enumerate(chunks):
                        cm = stats.tile([P, 1], f32, tag="cm")
                        nc.vector.tensor_reduce(cm[:], ps[:, :w], axis=AX.X, op=OP.max)
                        if k == 0:
                            nc.vector.tensor_copy(m[:], cm[:])
                        else:
                            nc.vector.tensor_max(m[:], m[:], cm[:])
                    nbias = stats.tile([P, 1], f32, tag="nb")
                    nc.scalar.mul(nbias[:], m[:], -INV_SQRT_D)
                    # exp((s - m)/32) -> A row (f32r) + fused row-sum
                    rsum = stats.tile([P, 1], f32, tag="rs")
                    for k, (ps, w, off) in enumerate(chunks):
                        cs = stats.tile([P, 1], f32, tag="cs")
                        nc.scalar.activation(
                            A[:, OFFS[jt] + off: OFFS[jt] + off + w],
                            ps[:, :w], AF.Exp,
                            bias=nbias[:], scale=INV_SQRT_D, accum_out=cs[:])
                        if k == 0:
                            nc.vector.tensor_copy(rsum[:], cs[:])
                        else:
                            nc.vector.tensor_add(rsum[:], rsum[:], cs[:])
                    rinv = stats.tile([P, 1], f32, tag="ri")
                    nc.vector.reciprocal(rinv[:], rsum[:])
                    # fold 1/rowsum into V: V'[jt] = V[jt] * rinv_j
                    nc.vector.tensor_scalar_mul(Vh[jt][:], Vh[jt][:], rinv[:])

            # ---- Phase 6: out half = A^T @ V' ----
            with tc.tile_pool(name="avps", bufs=4, space="PSUM") as avps, \
                 tc.tile_pool(name="p6s", bufs=3) as p6s:
                for it in range(NT):
                    ps = avps.tile([P, EH], f32, tag="av")
                    for jt in range(it + 1):
                        nc.tensor.matmul(
                            ps[:], A[:, OFFS[jt] + P * (it - jt): OFFS[jt] + P * (it - jt + 1)],
                            Vh[jt][:], start=(jt == 0), stop=(jt == it))
                    o_sb = p6s.tile([P, EH], f32, tag="oev")
                    nc.scalar.copy(o_sb[:], ps[:])
                    nc.sync.dma_start(out_ap[P * it:P * (it + 1), :], o_sb[:])

    nc.compile()
    return nc


_PROGRAM_CACHE = {}


def kernel(x, W_q, W_k, W_v):
    from concourse.bass_utils import run_bass_kernel_spmd

    x = np.asarray(x, dtype=np.float32)
    W_q = np.asarray(W_q, dtype=np.float32)
    W_k = np.asarray(W_k, dtype=np.float32)
    W_v = np.asarray(W_v, dtype=np.float32)

    if "nc" not in _PROGRAM_CACHE:
        _PROGRAM_CACHE["nc"] = build_program()
    nc = _PROGRAM_CACHE["nc"]

    in_maps = []
    for c in range(8):
        b, h = c // 2, c % 2
        in_maps.append({
            "xt": np.ascontiguousarray(x[b].T),
            "wq": W_q,
            "wk": W_k,
            "wvh": np.ascontiguousarray(W_v[:, h * EH:(h + 1) * EH]),
        })

    res = run_bass_kernel_spmd(nc, in_maps, core_ids=list(range(8)))
    out = np.empty((B, S, D), dtype=np.float32)
    for c in range(8):
        b, h = c // 2, c % 2
        out[b, :, h * EH:(h + 1) * EH] = res.results[c]["out"]
    return out
